# revision 7
# baseline (speedup 1.0000x reference)
"""Trainium2 Bass kernel for a 3-layer recurrent spiking net (LIF neurons).

Network (per timestep t, sequential over T):
    c1 = x_t @ W1.T + b1          [B,512]
    v1,s1 = LIF(v1, c1)           (v' = v + (c-v)/2; s = v'>=1; v = v'*(1-s))
    ir = s1 @ Wih.T + sr @ Whh.T + bih + bhh
    vr,sr = LIF(vr, ir)
    c2 = sr @ W2.T + b2           [B,2]
    v2,s2 = LIF(v2, c2)  -> output s2

Strategy: data-parallel over batch (32 -> 4 per core on 8 cores).  Per core a
3-deep staggered window pipeline over time (window = TS steps): during window w
the DVE processes LIF1 of subchunk w, LIF-r of subchunk w-1 (fused with LIF2 of
subchunk w-2 in the same instructions), while the PE streams the per-step
recurrent matmuls sr@Whh.T.  The batchable matmuls (C1 = X@W1.T, A = S1@Wih.T,
C2 = SR@W2.T) are done once per window into PSUM.

Feature-chunked layout everywhere: a 512-wide feature dim lives as 4 chunks of
128 on partitions, with (chunk, batch) or (chunk, batch, t) on the free dim.
"""

import numpy as np

import concourse.bacc as bacc
import concourse.mybir as mybir
from concourse.tile import TileContext
from concourse.bass_utils import run_bass_kernel_spmd

F32 = mybir.dt.float32
BF16 = mybir.dt.bfloat16

NCORES = 8
B = 32
BL = B // NCORES        # 4 batch rows per core
I = 128
H = 512
R = 512
O = 2
C = 4                   # feature chunks (512/128)
TS = 32                 # timesteps per window

_CACHE = {}


def build_kernel(T, mm_dtype=F32):
    """Build + compile the per-core Bass kernel for sequence length T."""
    assert T % TS == 0
    NW = T // TS
    nc = bacc.Bacc(trn_type="TRN2")

    mmd = mm_dtype
    # ---- DRAM I/O (per core) ----
    # xT[i, w*TS*BL + b*TS + t] = x[b, w*TS+t, i]
    xT = nc.dram_tensor("xT", [I, NW * TS * BL], mmd, kind="ExternalInput")
    w1t = nc.dram_tensor("w1t", [I, H], mmd, kind="ExternalInput")       # W1.T
    # wiht[p, kc*R + r] = Wih[r, kc*128+p]
    wiht = nc.dram_tensor("wiht", [128, C * R], mmd, kind="ExternalInput")
    whht = nc.dram_tensor("whht", [128, C * R], mmd, kind="ExternalInput")
    # w2t[p, kc*O + o] = W2[o, kc*128+p]
    w2t = nc.dram_tensor("w2t", [128, C * O], mmd, kind="ExternalInput")
    # y[o, w*TS*BL + b*TS + t] = s2[b, w*TS+t, o]
    yout = nc.dram_tensor("y", [O, T * BL], F32, kind="ExternalOutput")

    NB = TS * BL            # columns per block (= 128)
    SBLK = 9                # S-tile blocks: 4 x s1 | 4 x sr | s2

    with TileContext(nc) as tc:
        with (
            tc.tile_pool(name="wpool", bufs=1) as wpool,
            tc.tile_pool(name="state", bufs=1) as state,
            tc.tile_pool(name="xpool", bufs=3) as xpool,
            tc.tile_pool(name="asbp", bufs=2) as asbp,
            tc.tile_pool(name="pc1p", bufs=2, space="PSUM") as pc1p,
            tc.tile_pool(name="pacp", bufs=2, space="PSUM") as pacp,
            tc.tile_pool(name="pstepp", bufs=1, space="PSUM") as pstepp,
        ):
            # ---- static weights in SBUF ----
            w1t_sb = wpool.tile([I, H], mmd, tag="w1t")
            wiht_sb = wpool.tile([128, C * R], mmd, tag="wiht")
            whht_sb = wpool.tile([128, C * R], mmd, tag="whht")
            w2t_sb = wpool.tile([128, C * O], mmd, tag="w2t")
            nc.sync.dma_start(out=w1t_sb[:], in_=w1t[:])
            nc.sync.dma_start(out=wiht_sb[:], in_=wiht[:])
            nc.sync.dma_start(out=whht_sb[:], in_=whht[:])
            nc.sync.dma_start(out=w2t_sb[:], in_=w2t[:])

            ones_col = wpool.tile([128, 1], F32, tag="ones")
            nc.vector.memset(ones_col[:], 1.0)

            # ---- states / temps ----
            v1 = state.tile([128, C * BL], F32, tag="v1")          # (c,b)
            vr2 = state.tile([128, 5 * BL], F32, tag="vr2")        # (m,b); m=4 -> v2
            d1 = state.tile([128, C * BL], F32, tag="d1")
            dr = state.tile([128, 5 * BL], F32, tag="dr")
            uu = state.tile([128, 5 * BL], F32, tag="uu")
            m1 = state.tile([128, C * BL], F32, tag="m1")
            mr = state.tile([128, 5 * BL], F32, tag="mr")
            s2full = state.tile([O, T * BL], F32, tag="s2full")
            nc.vector.memset(v1[:], 0.0)
            nc.vector.memset(vr2[:], 0.0)

            # ---- S-tiles (spikes), 2 parities ----
            stiles = [
                state.tile([128, SBLK * NB], mmd, tag=f"stile{p}", name=f"stile{p}")
                for p in range(2)
            ]
            nc.vector.memset(stiles[0][:], 0.0)

            # ---- per-step psum accumulators (static pair) ----
            psteps = [
                pstepp.tile([128, 5 * BL], F32, tag=f"ps{i}", name=f"ps{i}")
                for i in range(2)
            ]
            nc.vector.memset(psteps[0][:], 0.0)
            nc.vector.memset(psteps[1][:], 0.0)

            ts_f32 = nc.vector.tensor_scalar
            stt = nc.vector.scalar_tensor_tensor
            AL = mybir.AluOpType

            for w in range(NW + 2):
                st_cur = stiles[w % 2]
                st_prev = stiles[1 - w % 2]
                stv_cur = st_cur[:].rearrange("p (s b t) -> p s b t", s=SBLK, b=BL, t=TS)
                stv_prev = st_prev[:].rearrange("p (s b t) -> p s b t", s=SBLK, b=BL, t=TS)

                # ================= PREP =================
                if w < NW:
                    xt = xpool.tile([I, NB], mmd, tag="xt")
                    nc.sync.dma_start(out=xt[:], in_=xT[:, w * NB:(w + 1) * NB])
                    pc1 = pc1p.tile([128, C * NB], F32, tag="pc1")
                    for c in range(C):
                        nc.tensor.matmul(
                            out=pc1[:, c * NB:(c + 1) * NB],
                            lhsT=w1t_sb[:, c * 128:(c + 1) * 128],
                            rhs=xt[:], start=True, stop=True)
                if 1 <= w <= NW + 1:
                    pac = pacp.tile([128, 5 * NB], F32, tag="pac")
                    if w <= NW:  # A(w-1) = S1(w-1) @ Wih.T
                        for m in range(C):
                            for kc in range(C):
                                nc.tensor.matmul(
                                    out=pac[:, m * NB:(m + 1) * NB],
                                    lhsT=wiht_sb[:, kc * R + m * 128: kc * R + (m + 1) * 128],
                                    rhs=st_prev[:, kc * NB:(kc + 1) * NB],
                                    start=(kc == 0), stop=(kc == C - 1))
                    if w >= 2:   # c2(w-2) = SR(w-2) @ W2.T; SR(w-2) was written by
                        # window w-1's LIF_r pass into st_prev blocks 4..7
                        for kc in range(C):
                            nc.tensor.matmul(
                                out=pac[0:O, 4 * NB:4 * NB + NB],
                                lhsT=w2t_sb[:, kc * O:(kc + 1) * O],
                                rhs=st_prev[:, (4 + kc) * NB:(5 + kc) * NB],
                                start=(kc == 0), stop=(kc == C - 1))
                    a_sb = asbp.tile([128, 5 * NB], F32, tag="a_sb")
                    if w <= NW:
                        nc.scalar.copy(out=a_sb[:, 0:4 * NB], in_=pac[:, 0:4 * NB])
                    if w >= 2:
                        # c2 lives on partitions 0..1 only; zero the rest so the
                        # fused LIF ops read deterministic values there.
                        nc.vector.memset(a_sb[:, 4 * NB:5 * NB], 0.0)
                        nc.scalar.copy(out=a_sb[0:O, 4 * NB:5 * NB],
                                       in_=pac[0:O, 4 * NB:5 * NB])
                    asv = a_sb[:].rearrange("p (m b t) -> p m b t", m=5, b=BL, t=TS)

                # ================= STEPS =================
                if w < NW:
                    vc1 = pc1[:].rearrange("p (c b t) -> p c b t", c=C, b=BL, t=TS)
                v1v = v1[:].rearrange("p (c b) -> p c b", c=C)
                m1v = m1[:].rearrange("p (c b) -> p c b", c=C)
                mrv = mr[:].rearrange("p (m b) -> p m b", m=5)
                d1v = d1[:].rearrange("p (c b) -> p c b", c=C)
                vrv = vr2[:].rearrange("p (m b) -> p m b", m=5)
                drv = dr[:].rearrange("p (m b) -> p m b", m=5)
                uuv = uu[:].rearrange("p (m b) -> p m b", m=5)

                for t in range(TS):
                    # ---- LIF1 on subchunk w ----
                    if w < NW:
                        c1s = vc1[:, :, :, t]
                        stt(d1v, v1v, -1.0, c1s, AL.mult, AL.add)        # d = c1 - v1
                        stt(v1v, d1v, 0.5, v1v, AL.mult, AL.add)         # v1 += d/2
                        ts_f32(stv_cur[:, 0:4, :, t], v1v, 1.0, None, AL.is_ge)
                        ts_f32(m1v, v1v, 1.0, None, AL.is_lt)
                        stt(v1v, m1v, 1.0, v1v, AL.mult, AL.mult)

                    # ---- recurrent layer on subchunk w-1 (+LIF2 on w-2) ----
                    if 1 <= w <= NW:
                        pstep = psteps[t % 2]
                        if t == 0:
                            src, col = stv_prev, TS - 1
                        else:
                            src, col = stv_cur, t - 1
                        for m in range(C):
                            for kc in range(C):
                                nc.tensor.matmul(
                                    out=pstep[:, m * BL:(m + 1) * BL],
                                    lhsT=whht_sb[:, kc * R + m * 128: kc * R + (m + 1) * 128],
                                    rhs=src[:, 4 + kc, :, col],
                                    start=(kc == 0), stop=(kc == C - 1))
                        nblk = 5 if w >= 2 else 4
                        ua = uuv[:, 0:nblk, :]
                        va = vrv[:, 0:nblk, :]
                        da = drv[:, 0:nblk, :]
                        stt(ua, pstep[:].rearrange("p (m b) -> p m b", m=5)[:, 0:nblk, :],
                            0.0, asv[:, 0:nblk, :, t], AL.add, AL.add)   # u = W + A
                        stt(da, va, -1.0, ua, AL.mult, AL.add)           # d = u - v
                        stt(va, da, 0.5, va, AL.mult, AL.add)            # v += d/2
                        ts_f32(stv_cur[:, 4:4 + nblk, :, t], va, 1.0, None, AL.is_ge)
                        ma = mrv[:, 0:nblk, :]
                        ts_f32(ma, va, 1.0, None, AL.is_lt)
                        stt(va, ma, 1.0, va, AL.mult, AL.mult)
                    elif w == NW + 1:
                        # LIF2 only, on subchunk w-2 = NW-1
                        va = vrv[:, 4, :]
                        da = drv[:, 4, :]
                        stt(da, va, -1.0, asv[:, 4, :, t], AL.mult, AL.add)
                        stt(va, da, 0.5, va, AL.mult, AL.add)
                        ts_f32(stv_cur[:, 8, :, t], va, 1.0, None, AL.is_ge)
                        ma = mrv[:, 4, :]
                        ts_f32(ma, va, 1.0, None, AL.is_lt)
                        stt(va, ma, 1.0, va, AL.mult, AL.mult)

                # ---- export s2(w-2) ----
                if w >= 2:
                    nc.vector.tensor_copy(
                        out=s2full[:, (w - 2) * NB:(w - 1) * NB],
                        in_=st_cur[0:O, 8 * NB:9 * NB])

            nc.sync.dma_start(out=yout[:], in_=s2full[:])

    nc.compile()
    return nc


def _np_dt(mm_dtype):
    if mm_dtype == BF16:
        import ml_dtypes
        return ml_dtypes.bfloat16
    return np.float32


def _prep_core_inputs(x_core, W1, Wih, Whh, W2, T, mm_dtype):
    npdt = _np_dt(mm_dtype)
    NW = T // TS
    xr = np.ascontiguousarray(x_core.reshape(BL, NW, TS, I).transpose(3, 1, 0, 2))
    return {
        "xT": xr.reshape(I, NW * TS * BL).astype(npdt),
        "w1t": np.ascontiguousarray(W1.T).astype(npdt),
        "wiht": np.ascontiguousarray(
            Wih.T.reshape(C, 128, R).transpose(1, 0, 2)).reshape(128, C * R).astype(npdt),
        "whht": np.ascontiguousarray(
            Whh.T.reshape(C, 128, R).transpose(1, 0, 2)).reshape(128, C * R).astype(npdt),
        "w2t": np.ascontiguousarray(
            W2.T.reshape(C, 128, O).transpose(1, 0, 2)).reshape(128, C * O).astype(npdt),
    }


def run(x, W1, b1, Wih, bih, Whh, bhh, W2, b2, mm_dtype=F32, trace=False):
    x = np.asarray(x); W1 = np.asarray(W1); Wih = np.asarray(Wih)
    Whh = np.asarray(Whh); W2 = np.asarray(W2)
    for b_ in (b1, bih, bhh, b2):
        assert not np.any(np.asarray(b_)), "nonzero biases unsupported"
    Bfull, T, _ = x.shape
    assert Bfull == B
    key = (T, mm_dtype)
    if key not in _CACHE:
        _CACHE[key] = build_kernel(T, mm_dtype)
    nc = _CACHE[key]
    in_maps = [
        _prep_core_inputs(x[c * BL:(c + 1) * BL], W1, Wih, Whh, W2, T, mm_dtype)
        for c in range(NCORES)
    ]
    res = run_bass_kernel_spmd(nc, in_maps, core_ids=list(range(NCORES)), trace=trace)
    NW = T // TS
    outs = []
    for c in range(NCORES):
        y = res.results[c]["y"]  # [O, T*BL]
        yl = y.reshape(O, NW, BL, TS).transpose(2, 1, 3, 0).reshape(BL, T, O)
        outs.append(yl)
    return np.concatenate(outs, axis=0).astype(np.float32), res


def kernel(**inputs):
    out, _ = run(**inputs)
    return out
